# revision 25
# baseline (speedup 1.0000x reference)
"""Trainium2 Bass kernel for BSplineLayer: y = BSpline(knots, coeffs, k=3)((x - min(x)) / (max(x) - min(x) + 1e-8)).

Because the reference clips the de Boor interval index to [k, n-1] = [3, 3]
(n = len(knots) - k - 1 = 4 basis functions), the whole layer reduces to a
single cubic polynomial P(xn) evaluated everywhere, with coefficients that
depend only on knots/coeffs.  The device computes the global min/max
(AllReduce(max) of (max, -min) across the 8 cores), folds the normalization
into composed coefficients q_i, and evaluates the cubic in raw x as
y = (x^2 + q1/q3) * (q3*x + q2) + (q0 - q2*q1/q3): one stats-free ACT Square
(pre-computed while the collective is in flight), one DVE tensor_scalar (2x
mode), one DVE scalar_tensor_tensor, one ACT affine.

Per-core layout: x is sharded row-wise (1024 rows/core), kept SBUF-resident
(16 MiB) so HBM traffic is one read + one write of the shard.  Phase 1
min/max reduces run on DVE as tiles stream in (early tiles split so the
reduces start at first-landing); a dependency-free warm-up collective
absorbs the ncfw setup + core-skew cost before the real one.
"""

import sys

sys.path.insert(0, "/opt/trn_rl_repo")

import numpy as np

N_CORES = 8
ROWS, COLS = 8192, 4096
R_CORE = ROWS // N_CORES          # 1024 rows per core
P = 128                           # SBUF partitions
N_TILES = R_CORE // P             # 8 tiles of [128, 4096] per core
CHUNK = 2048                      # phase-2 free-dim chunk
DEGREE = 3

_CACHE = {}


def _expand_cubic(knots: np.ndarray, coeffs: np.ndarray) -> np.ndarray:
    """Expand de Boor at interval m=3 into monomial coeffs [a0, a1, a2, a3] (float64)."""
    t = np.asarray(knots, dtype=np.float64)
    c = np.asarray(coeffs, dtype=np.float64)
    k = DEGREE
    m = k  # reference clips searchsorted result to [k, n-1] with n-1 == k
    pm = np.polynomial.polynomial
    d = [np.array([c[m - k + j]], dtype=np.float64) for j in range(k + 1)]
    for r in range(1, k + 1):
        for j in range(k, r - 1, -1):
            tl = t[m - k + j]
            tr = t[m + j + 1 - r]
            inv = 1.0 / (tr - tl)
            alpha = np.array([-tl * inv, inv])
            one_m = np.array([1.0 + tl * inv, -inv])
            d[j] = pm.polyadd(pm.polymul(one_m, d[j - 1]), pm.polymul(alpha, d[j]))
    a = np.zeros(4, dtype=np.float64)
    a[: len(d[k])] = d[k]
    return a


def _build_program():
    import concourse.bass as bass
    import concourse.tile as tile
    from concourse import bacc, bass_isa, mybir

    dt = mybir.dt.float32
    OP = mybir.AluOpType
    AX = mybir.AxisListType
    AF = mybir.ActivationFunctionType

    nc = bacc.Bacc("TRN2", target_bir_lowering=False, debug=False, num_devices=N_CORES)
    x_ext = nc.declare_dram_parameter("x", [R_CORE, COLS], dt, isOutput=False)
    ac_ext = nc.declare_dram_parameter("ac", [1, 4], dt, isOutput=False)
    y_ext = nc.declare_dram_parameter("y", [R_CORE, COLS], dt, isOutput=True)

    with tile.TileContext(nc) as tc:
        with (
            tc.tile_pool(name="xp", bufs=1) as xp,
            tc.tile_pool(name="xsqp", bufs=6) as xsqp,
            tc.tile_pool(name="wp", bufs=2) as wp,
            tc.tile_pool(name="small", bufs=1) as small,
            tc.tile_pool(name="dram", bufs=1, space="DRAM") as dram,
        ):
            # Warm the collective path (ncfw queue/ring setup + core-skew sync)
            # concurrently with the phase-1 loads so the real AllGather is
            # cheap.  Gathers an uninitialized DRAM word on purpose: zero
            # dependencies means the gpsimd stream enqueues it immediately.
            warm_in = dram.tile([1, 2], dt)
            warm_out = dram.tile([1, 2], dt)
            nc.gpsimd.collective_compute(
                "AllReduce", OP.max,
                replica_groups=[list(range(N_CORES))],
                ins=[warm_in[:].opt()], outs=[warm_out[:].opt()],
            )

            # ---------------- phase 1: load + local min/max ----------------
            # Tile 0 is loaded in quarters so the first reduce starts as soon
            # as the first 512 KiB lands instead of after the full 2 MiB.
            N_RED = N_TILES + 4  # tile0 in 4 pieces + tile1 in 2 + 6 whole
            xts = []
            rmax8 = small.tile([P, N_RED], dt)
            rmin8 = small.tile([P, N_RED], dt)
            n_chunks = COLS // CHUNK
            XSQ_PREP = 6  # == xsqp bufs; more would deadlock ACT's in-order stream
            xsqs = []

            def chunk_ap(ci):
                t, h = divmod(ci, n_chunks)
                return xts[t][:, h * CHUNK:(h + 1) * CHUNK]

            def emit_square(ci):
                # Stats-free x^2 on ACT; the first XSQ_PREP are emitted during
                # phase 1 so ACT runs ahead of the collective.
                xsq = xsqp.tile([P, CHUNK], dt, tag="xsq")
                xsqs.append(xsq)
                nc.scalar.activation(xsq[:], chunk_ap(ci), AF.Square,
                                     bias=0.0, scale=1.0)

            QC = COLS // 4
            ri = 0
            for t in range(N_TILES):
                xt = xp.tile([P, COLS], dt, tag=f"x{t}")
                xts.append(xt)
                nq = 4 if t == 0 else (2 if t == 1 else 1)  # early tiles in
                QT = COLS // nq                             # pieces: reduces
                for qq in range(nq):                        # start sooner
                    nc.sync.dma_start(out=xt[:, qq * QT:(qq + 1) * QT],
                                      in_=x_ext[t * P:(t + 1) * P, qq * QT:(qq + 1) * QT])
                for qq in range(nq):
                    xq = xt[:, qq * QT:(qq + 1) * QT]
                    nc.vector.tensor_reduce(rmax8[:, ri:ri + 1], xq, axis=AX.X, op=OP.max)
                    nc.vector.tensor_reduce(rmin8[:, ri:ri + 1], xq, axis=AX.X, op=OP.min)
                    ri += 1
            for ci in range(XSQ_PREP):
                emit_square(ci)

            pk = small.tile([P, 2], dt)
            nc.vector.tensor_reduce(pk[:, 0:1], rmax8[:], axis=AX.X, op=OP.max)
            rmn = small.tile([P, 1], dt)
            nc.vector.tensor_reduce(rmn[:], rmin8[:], axis=AX.X, op=OP.min)
            nc.vector.tensor_scalar_mul(pk[:, 1:2], rmn[:], -1.0)

            # cross-partition: every partition gets (local_max, -local_min)
            par = small.tile([P, 2], dt)
            nc.gpsimd.partition_all_reduce(par[:], pk[:], channels=P,
                                           reduce_op=bass_isa.ReduceOp.max)

            # cross-core: AllReduce(max) of the pair
            cc_in = dram.tile([1, 2], dt)
            cc_out = dram.tile([1, 2], dt)
            nc.sync.dma_start(out=cc_in[:], in_=par[0:1, 0:2])
            nc.gpsimd.collective_compute(
                "AllReduce", OP.max,
                replica_groups=[list(range(N_CORES))],
                ins=[cc_in[:].opt()], outs=[cc_out[:].opt()],
            )
            GG = small.tile([P, 2], dt)
            nc.sync.dma_start(out=GG[:], in_=cc_out[:].partition_broadcast(P))

            # host constants in: ac = [e2a=a2/a3, e1a=a1/a3, a3, a0]
            ac_sb = small.tile([1, 4], dt)
            nc.sync.dma_start(out=ac_sb[:], in_=ac_ext[:])
            AC = small.tile([P, 4], dt)
            nc.gpsimd.partition_broadcast(AC[:], ac_sb[:])
            e2a, e1a, a3c, a0c = (AC[:, i:i + 1] for i in range(4))

            # ------- device scalars: normalization + composed coefficients -------
            # s = 1/(gmax + gnm + eps); b = gnm*s    (gnm = -gmin)
            # y = P(s*x + b) = ((x + d2)*x + d1)*x*q3 + q0
            #   d2 = (3b + e2a)*d        (d = 1/s)
            #   d1 = ((3b + 2*e2a)*b + e1a)*d^2
            #   q3 = a3*s^3
            #   q0 = a3*(b + e2a)*b^2 + a3*e1a*b + a0
            cf = small.tile([P, 8], dt)
            d2c, d1c, q3c, q0c, g1c, g2c, alc = (cf[:, i:i + 1] for i in range(7))
            tmp = small.tile([P, 10], dt)
            dd, s_, b_, u, v, w, s2, p_, de_, _sp = (tmp[:, i:i + 1] for i in range(10))

            nc.vector.scalar_tensor_tensor(dd, GG[:, 0:1], 1e-8, GG[:, 1:2],
                                           op0=OP.add, op1=OP.add)      # d = range+eps
            nc.vector.reciprocal(s_, dd)
            nc.vector.tensor_tensor(b_, GG[:, 1:2], s_, op=OP.mult)     # b = gnm*s

            nc.vector.tensor_scalar_mul(u, b_, 3.0)                     # u = 3b
            nc.vector.tensor_tensor(v, u, e2a, op=OP.add)               # v = 3b+e2a
            nc.vector.tensor_tensor(d2c, v, dd, op=OP.mult)             # d2

            nc.vector.scalar_tensor_tensor(w, e2a, 2.0, u, op0=OP.mult, op1=OP.add)  # w = 2e2a+3b
            nc.vector.tensor_tensor(w, w, b_, op=OP.mult)
            nc.vector.tensor_tensor(w, w, e1a, op=OP.add)               # (3b+2e2a)b+e1a
            nc.vector.tensor_tensor(v, dd, dd, op=OP.mult)              # v = d^2
            nc.vector.tensor_tensor(d1c, w, v, op=OP.mult)              # d1

            nc.vector.tensor_tensor(s2, s_, s_, op=OP.mult)
            nc.vector.tensor_tensor(u, s2, s_, op=OP.mult)              # s^3
            nc.vector.tensor_tensor(q3c, u, a3c, op=OP.mult)            # q3

            nc.vector.tensor_tensor(p_, b_, e2a, op=OP.add)             # b+e2a
            nc.vector.tensor_tensor(g2c, p_, a3c, op=OP.mult)           # g2 = a3*(b+e2a)
            nc.vector.tensor_tensor(p_, p_, b_, op=OP.mult)
            nc.vector.tensor_tensor(p_, p_, e1a, op=OP.add)             # (b+e2a)b+e1a
            nc.vector.tensor_tensor(p_, p_, b_, op=OP.mult)
            nc.vector.tensor_tensor(p_, p_, a3c, op=OP.mult)            # a3*(...)
            nc.vector.tensor_tensor(q0c, p_, a0c, op=OP.add)            # q0

            # raw-x evaluation constants: y = (x^2 + alpha)*(q3*x + q2) + delta
            # with alpha = q1/q3 = d1, q2 = d2*q3, delta = q0 - q2*d1.
            nc.vector.tensor_tensor(g1c, d2c, q3c, op=OP.mult)          # q2 (in col 4)
            nc.vector.tensor_tensor(de_, g1c, d1c, op=OP.mult)
            nc.vector.tensor_tensor(de_, q0c, de_, op=OP.subtract)      # delta

            # ACT-owned copy of delta: the phase-2 Identity then waits on at
            # most one foreign semaphore (the wait-slot limit workaround).
            actsb = small.tile([P, 1], dt)
            nc.scalar.copy(actsb[:, 0:1], de_)                          # delta

            # ---------------- phase 2: evaluate + store ----------------
            # Uniform raw-x form: xsq = Square(x) (stats-free, so ACT runs
            # ahead during phase 1 / the collective, bounded by the xsq pool
            # depth); t1 = q3*x + q2 (DVE TS @2x); u = (xsq + alpha)*t1
            # (DVE STT, in place over xsq); y = u + delta (ACT).
            total_chunks = N_TILES * n_chunks
            for ci in range(total_chunks):
                t, h = divmod(ci, n_chunks)
                xc = chunk_ap(ci)
                xsq = xsqs[ci]
                t1 = wp.tile([P, CHUNK], dt, tag="t1")
                nc.vector.tensor_scalar(t1[:], xc, q3c, g1c,
                                        op0=OP.mult, op1=OP.add)
                nc.vector.scalar_tensor_tensor(xsq[:], xsq[:], d1c, t1[:],
                                               op0=OP.add, op1=OP.mult)
                nc.scalar.activation(xc, xsq[:], AF.Identity,
                                     bias=actsb[:, 0:1], scale=1.0)
                if ci + XSQ_PREP < total_chunks:
                    emit_square(ci + XSQ_PREP)
                nc.sync.dma_start(
                    out=y_ext[t * P:(t + 1) * P, h * CHUNK:(h + 1) * CHUNK],
                    in_=xc)

    nc.compile()
    return nc


def kernel(x: np.ndarray, knots: np.ndarray, coeffs: np.ndarray) -> np.ndarray:
    from concourse.bass_utils import run_bass_kernel_spmd

    x = np.ascontiguousarray(np.asarray(x, dtype=np.float32))
    assert x.shape == (ROWS, COLS), x.shape

    a = _expand_cubic(knots, coeffs)
    a3 = a[3] if abs(a[3]) > 1e-30 else 1e-30
    ac = np.array([[a[2] / a3, a[1] / a3, a3, a[0]]], dtype=np.float32)

    if "nc" not in _CACHE:
        _CACHE["nc"] = _build_program()
    nc = _CACHE["nc"]

    shards = [x[i * R_CORE:(i + 1) * R_CORE] for i in range(N_CORES)]
    in_maps = [{"x": s, "ac": ac} for s in shards]

    import os
    trace = bool(int(os.environ.get("KERNEL_TRACE", "0")))
    res = run_bass_kernel_spmd(nc, in_maps, core_ids=list(range(N_CORES)),
                               trace=trace)
    if trace and res.exec_time_ns is not None:
        print(f"HW exec time: {res.exec_time_ns} ns")
        _CACHE["last_exec_time_ns"] = res.exec_time_ns
        _CACHE["last_trace"] = res.instructions_and_trace

    out = np.empty((ROWS, COLS), dtype=np.float32)
    for i in range(N_CORES):
        out[i * R_CORE:(i + 1) * R_CORE] = res.results[i]["y"]
    return out
